# revision 16
# baseline (speedup 1.0000x reference)
"""Trainium2 Bass kernel for nn_FAM1 (FSM + modulated deformable conv block).

8 cores, data-parallel: core i handles batch b=i//4, rows [40*(i%4), +40).
The bilinear DCN gather is computed exactly as a dense 5x5 window of shifted
reads weighted by hat-products:
  val = sum_{a,b} max(0,1-|dy-a|) * max(0,1-|dx-b|) * mask * x[p + a*W + b]
(hats vanish outside the active 2x2 corners; |offsets| < 2 so 5x5 is exact).
All per-pixel tensors live on a padded 168-wide grid so every vector op is a
flat contiguous bf16 stream (DVE 2x mode).  (d,k)-level weight fields are
expanded to the (d,c) 128-partition layout with a replicating SBUF->SBUF DMA.
"""
import sys
if '/opt/trn_rl_repo' not in sys.path:
    sys.path.insert(0, '/opt/trn_rl_repo')

from contextlib import ExitStack

import numpy as np
import ml_dtypes

import concourse.bass as bass
import concourse.bacc as bacc
import concourse.tile as tile
from concourse import mybir
from concourse.bass_utils import run_bass_kernel_spmd

BF = ml_dtypes.bfloat16
F32 = mybir.dt.float32
BF16 = mybir.dt.bfloat16
AF = mybir.ActivationFunctionType
OP = mybir.AluOpType

B, C1, C2, H, W = 2, 256, 128, 160, 160
DG, K, KK = 8, 3, 9
SH = 40                  # stripe rows per core
XR = 48                  # xs rows (stripe + 4 halo each side)
PW = 168                 # padded grid pitch (4 + 160 + 4)
ER = 42                  # extended rows (stripe + 1 halo each side)
OFR = 44                 # off_feat buffer rows (ER + 1 zero row each side)
CH = 10                  # chunk rows
NCH = SH // CH
FCH = CH * PW            # 1680
AY = (-2, -1, 0, 1, 2)
AX = (-2, -1, 0, 1, 2)
SUB = 2 * PW             # 336: om/einsum psum sub-chunk (2 padded rows)

_CACHE = {}


def _build_program():
    nc = bacc.Bacc("TRN2", target_bir_lowering=False, debug=False)
    for v in (-1.0, 2.0, 3.0):
        t = nc.alloc_sbuf_tensor(f"const-f32-{v}", [128, 1], F32)
        nc.gpsimd.memset(t.ap(), v)
        nc.const_aps.aps[(F32, v)] = t.ap()
    dp = nc.declare_dram_parameter
    xs0 = dp("xs0", [C2, XR * PW], BF16, isOutput=False)
    xs1 = dp("xs1", [C2, XR * PW], BF16, isOutput=False)
    fl = dp("fl", [C1, ER * W], F32, isOutput=False)
    watten = dp("watten", [C1, C1], F32, isOutput=False)
    wconv = dp("wconv", [C1, C2], F32, isOutput=False)
    wofffa = dp("wofffa", [C2, C2], BF16, isOutput=False)
    wofffs = dp("wofffs", [C2, C2], BF16, isOutput=False)
    wom = dp("wom", [C2, 9 * 216], BF16, isOutput=False)
    wdcn = dp("wdcn", [C2, 9 * C2], BF16, isOutput=False)
    dcnb = dp("dcnb", [C2, 1], F32, isOutput=False)
    ombp = dp("ombp", [216, 1], F32, isOutput=False)
    gsel = dp("gsel", [C2, 4], F32, isOutput=False)
    out_pad = dp("out_pad", [C2, SH * PW], F32, isOutput=True)

    farm32 = nc.dram_tensor("farm32", [C2, SH * PW], F32)
    farmbf = nc.dram_tensor("farmbf", [C2, ER * W], BF16)
    gap_in = nc.dram_tensor("gap_in", [C2, 4], F32)
    gap_out = nc.dram_tensor("gap_out", [C2, 4], F32, addr_space="Shared")
    groups = [list(range(8))]

    with tile.TileContext(nc) as tc, ExitStack() as ctx:
        wpool = ctx.enter_context(tc.tile_pool(name="wts", bufs=1))
        big = ctx.enter_context(tc.tile_pool(name="big", bufs=1))

        # ---- weights ----
        w_at0 = wpool.tile([C2, C1], F32, tag="w_at0")
        w_at1 = wpool.tile([C2, C1], F32, tag="w_at1")
        nc.sync.dma_start(out=w_at0[:], in_=watten[0:C2, :])
        nc.sync.dma_start(out=w_at1[:], in_=watten[C2:C1, :])
        w_cv0 = wpool.tile([C2, C2], F32, tag="w_cv0")
        w_cv1 = wpool.tile([C2, C2], F32, tag="w_cv1")
        nc.sync.dma_start(out=w_cv0[:], in_=wconv[0:C2, :])
        nc.sync.dma_start(out=w_cv1[:], in_=wconv[C2:C1, :])
        w_oa = wpool.tile([C2, C2], BF16, tag="w_oa")
        nc.sync.dma_start(out=w_oa[:], in_=wofffa[:])
        w_os = wpool.tile([C2, C2], BF16, tag="w_os")
        nc.sync.dma_start(out=w_os[:], in_=wofffs[:])
        w_om = wpool.tile([C2, 9 * 216], BF16, tag="w_om")
        nc.sync.dma_start(out=w_om[:], in_=wom[:])
        w_dc = wpool.tile([C2, 9 * C2], BF16, tag="w_dc")
        nc.sync.dma_start(out=w_dc[:], in_=wdcn[:])
        b_dc = wpool.tile([C2, 1], F32, tag="b_dc")
        nc.sync.dma_start(out=b_dc[:], in_=dcnb[:])
        b_om = wpool.tile([72, 3], F32, tag="b_om")
        nc.sync.dma_start(out=b_om[:, 0:1], in_=ombp[0:72, :])
        nc.sync.dma_start(out=b_om[:, 1:2], in_=ombp[72:144, :])
        nc.sync.dma_start(out=b_om[:, 2:3], in_=ombp[144:216, :])

        xs0t = big.tile([C2, XR * PW], BF16, tag="xs0t")
        nc.sync.dma_start(out=xs0t[:], in_=xs0[:])
        xs1t = big.tile([C2, XR * PW], BF16, tag="xs1t")
        nc.sync.dma_start(out=xs1t[:], in_=xs1[:])
        off = big.tile([C2, OFR * PW + 8], BF16, tag="off")
        nc.vector.memset(off[:], 0.0)

        # ---- phases 0-2 (scoped pools, freed afterwards) ----
        NS1 = 3 * W  # 480
        with tc.tile_pool(name="flp", bufs=1) as flp, \
             tc.tile_pool(name="st12", bufs=2) as st12, \
             tc.tile_pool(name="ps12", bufs=2, space=bass.MemorySpace.PSUM) as ps12:
            fla = flp.tile([C2, ER * W], F32, tag="fla")
            flb = flp.tile([C2, ER * W], F32, tag="flb")
            nc.sync.dma_start(out=fla[:], in_=fl[0:C2, :])
            nc.sync.dma_start(out=flb[:], in_=fl[C2:C1, :])
            gp = wpool.tile([C2, 2], F32, tag="gp")
            gap_sb = wpool.tile([C2, 4], F32, tag="gap_sb")
            gsl0 = wpool.tile([C2, 4], F32, tag="gsl0")
            nc.sync.dma_start(out=gsl0[:], in_=gsel[:])
            gsl = wpool.tile([C2, 4], F32, tag="gsl")
            nc.vector.tensor_copy(gsl[:], gsl0[:])
            nc.vector.tensor_reduce(out=gp[:, 0:1], in_=fla[:, W:(ER - 1) * W],
                                    axis=mybir.AxisListType.X, op=OP.add)
            nc.vector.tensor_reduce(out=gp[:, 1:2], in_=flb[:, W:(ER - 1) * W],
                                    axis=mybir.AxisListType.X, op=OP.add)
            # zero/keep own-batch column pair via per-core mask, 8-core allreduce
            nc.vector.tensor_tensor(out=gap_sb[:].rearrange("p (a t) -> p a t", a=2),
                                    in0=gp[:].unsqueeze(1)
                                    .broadcast_to([C2, 2, 2]),
                                    in1=gsl[:].rearrange("p (a t) -> p a t", a=2),
                                    op=OP.mult)
            nc.gpsimd.dma_start(out=gap_in[:], in_=gap_sb[:])
            nc.gpsimd.collective_compute(
                "AllReduce", OP.add, replica_groups=groups,
                ins=[gap_in[:]], outs=[gap_out[:]])
            g4 = wpool.tile([C2, 4], F32, tag="g4")
            nc.gpsimd.dma_start(out=g4[:], in_=gap_out[:])
            g_sb = wpool.tile([C2, 2], F32, tag="g_sb")
            nc.vector.tensor_tensor(out=g_sb[:], in0=g4[:, 0:2], in1=g4[:, 2:4],
                                    op=OP.add)
            tc.strict_bb_all_engine_barrier()

            s1 = wpool.tile([C2, 2], F32, tag="s1")
            for m in range(2):
                p_at = ps12.tile([C2, 1], F32, tag="p_at")
                w_m = (w_at0, w_at1)
                for t in range(2):
                    nc.tensor.matmul(p_at[:],
                                     w_m[t][:, m * C2:(m + 1) * C2],
                                     g_sb[:, t:t + 1],
                                     start=(t == 0), stop=(t == 1))
                nc.scalar.activation(s1[:, m:m + 1], p_at[:], AF.Sigmoid)
            nc.vector.tensor_scalar(out=s1[:], in0=s1[:], scalar1=1.0,
                                    scalar2=None, op0=OP.add)

            # feat_arm
            nc.scalar.activation(fla[:], fla[:], AF.Copy, scale=s1[:, 0:1])
            nc.scalar.activation(flb[:], flb[:], AF.Copy, scale=s1[:, 1:2])
            for s in range(ER // 3):
                p_fa = ps12.tile([C2, NS1], F32, tag="p_fa")
                sl = bass.ts(s, NS1)
                nc.tensor.matmul(p_fa[:], w_cv0[:], fla[:, sl],
                                 start=True, stop=False)
                nc.tensor.matmul(p_fa[:], w_cv1[:], flb[:, sl],
                                 start=False, stop=True)
                fab = st12.tile([C2, NS1], BF16, tag="fab")
                nc.vector.tensor_copy(fab[:], p_fa[:])
                nc.sync.dma_start(out=farmbf[:, sl], in_=fab[:])
                r0, r1 = 3 * s, 3 * s + 3
                ri0, ri1 = max(r0, 1), min(r1, ER - 1)
                if ri1 > ri0:
                    fa32 = st12.tile([C2, NS1], F32, tag="fa32")
                    nc.scalar.activation(fa32[:], p_fa[:], AF.Copy)
                    nr = ri1 - ri0
                    src = fa32[:, (ri0 - r0) * W:(ri0 - r0 + nr) * W] \
                        .rearrange("p (r w) -> p r w", r=nr)
                    dst = farm32[:, :].rearrange("p (r w) -> p r w", w=PW)[
                        :, ri0 - 1:ri1 - 1, 4:4 + W]
                    nc.sync.dma_start(out=dst, in_=src)

            # off_feat: buffer rows 1..43 = ext rows 0..42, zeros elsewhere
            for s in range(ER // 3):
                p_of = ps12.tile([C2, NS1], F32, tag="p_of")
                fab2 = st12.tile([C2, NS1], BF16, tag="fab2")
                nc.sync.dma_start(out=fab2[:], in_=farmbf[:, bass.ts(s, NS1)])
                nc.tensor.matmul(p_of[:], w_oa[:], fab2[:],
                                 start=True, stop=False)
                rhs2 = xs0t[:, :].rearrange("p (r w) -> p r w", w=PW)[
                    :, 3 + 3 * s:6 + 3 * s, 4:4 + W]
                nc.tensor.matmul(p_of[:], w_os[:], rhs2,
                                 start=False, stop=True)
                dst = off[:, 0:OFR * PW].rearrange("p (r w) -> p r w", w=PW)[
                    :, 1 + 3 * s:4 + 3 * s, 4:4 + W]
                src_r = p_of[:].rearrange("p (r w) -> p r w", r=3)
                nc.vector.tensor_copy(dst, src_r)

        # ---- phase 3 ----
        with tc.tile_pool(name="chp", bufs=1) as chp, \
             tc.tile_pool(name="hey", bufs=2) as hey, \
             tc.tile_pool(name="hex", bufs=1) as hex_, \
             tc.tile_pool(name="mac", bufs=1) as mac, \
             tc.tile_pool(name="st3", bufs=2) as st3, \
             tc.tile_pool(name="ps3", bufs=1, space=bass.MemorySpace.PSUM) as ps3, \
             tc.tile_pool(name="pd", bufs=1, space=bass.MemorySpace.PSUM) as pdp:
            for chk in range(NCH):
                r0 = chk * CH
                dy_f = chp.tile([72, FCH], BF16, tag="dy_f")
                dx_f = chp.tile([72, FCH], BF16, tag="dx_f")
                msk = chp.tile([72, FCH], BF16, tag="msk")
                for s in range(CH // 2):
                    orow = r0 + 2 * s
                    pY = ps3.tile([72, SUB], F32, tag="pY")
                    pX = ps3.tile([72, SUB], F32, tag="pX")
                    pM = ps3.tile([72, SUB], F32, tag="pM")
                    for i in range(9):
                        ky, kx = i // 3 - 1, i % 3 - 1
                        base = (orow + 2 + ky) * PW + kx
                        rhs = off[:, base:base + SUB]
                        nc.tensor.matmul(pY[:],
                                         w_om[:, i * 216:i * 216 + 72], rhs,
                                         start=(i == 0), stop=(i == 8))
                        nc.tensor.matmul(pX[:],
                                         w_om[:, i * 216 + 72:i * 216 + 144], rhs,
                                         start=(i == 0), stop=(i == 8))
                        nc.tensor.matmul(pM[:],
                                         w_om[:, i * 216 + 144:(i + 1) * 216], rhs,
                                         start=(i == 0), stop=(i == 8))
                    sl = bass.ts(s, SUB)
                    nc.scalar.activation(dy_f[:, sl], pY[:], AF.Identity,
                                         bias=b_om[:, 0:1])
                    nc.scalar.activation(dx_f[:, sl], pX[:], AF.Identity,
                                         bias=b_om[:, 1:2])
                    nc.scalar.activation(msk[:, sl], pM[:], AF.Sigmoid,
                                         bias=b_om[:, 2:3])

                h72 = chp.tile([72, 10 * FCH], BF16, tag="h72")
                tmp = chp.tile([72, FCH], BF16, tag="tmp")
                tmp2 = chp.tile([72, FCH], BF16, tag="tmp2")
                # hat(t-a) = min(relu(1-(t-a)), relu(1+(t-a)))
                for ai, a in enumerate(AY):
                    nc.scalar.activation(tmp[:], dy_f[:], AF.Relu,
                                         bias=1.0 + a, scale=-1.0)
                    nc.scalar.activation(tmp2[:], dy_f[:], AF.Relu,
                                         bias=1.0 - a, scale=1.0)
                    nc.vector.tensor_tensor(out=tmp[:], in0=tmp[:], in1=tmp2[:],
                                            op=OP.min)
                    nc.vector.tensor_tensor(out=h72[:, bass.ts(ai, FCH)],
                                            in0=tmp[:], in1=msk[:], op=OP.mult)
                for bi, bx in enumerate(AX):
                    nc.scalar.activation(tmp[:], dx_f[:], AF.Relu,
                                         bias=1.0 + bx, scale=-1.0)
                    nc.scalar.activation(tmp2[:], dx_f[:], AF.Relu,
                                         bias=1.0 - bx, scale=1.0)
                    nc.vector.tensor_tensor(out=h72[:, bass.ts(5 + bi, FCH)],
                                            in0=tmp[:], in1=tmp2[:], op=OP.min)

                pd = []
                for i in range(CH // 2):
                    pdt = pdp.tile([C2, SUB], F32, tag=f"pd{i}", name=f"pd{i}")
                    pd.append(pdt)
                for k in range(KK):
                    ky, kx = k // 3 - 1, k % 3 - 1
                    hEy = hey.tile([C2, 5 * FCH], BF16, tag="hEy")
                    repy = h72[8 * k:8 * k + 8, 0:5 * FCH].unsqueeze(1) \
                        .broadcast_to([8, 16, 5 * FCH])
                    nc.sync.dma_start(out=hEy[:], in_=repy)
                    hEx = hex_.tile([C2, 5 * FCH], BF16, tag="hEx")
                    repx = h72[8 * k:8 * k + 8, 5 * FCH:10 * FCH].unsqueeze(1) \
                        .broadcast_to([8, 16, 5 * FCH])
                    nc.sync.dma_start(out=hEx[:], in_=repx)

                    S = mac.tile([C2, FCH], BF16, tag="S")
                    Y = mac.tile([C2, FCH], BF16, tag="Y")
                    t1 = mac.tile([C2, FCH], BF16, tag="t1")
                    for bi, bx in enumerate(AX):
                        sh = kx + bx
                        xs_t, xbase = (xs0t, 0) if (sh % 2 == 0) else (xs1t, 1)
                        for ai, a in enumerate(AY):
                            o0 = (r0 + 4 + ky + a) * PW + xbase + sh
                            xsl = xs_t[:, o0:o0 + FCH]
                            dst = Y if ai == 0 else t1
                            nc.vector.tensor_tensor(
                                out=dst[:], in0=hEy[:, bass.ts(ai, FCH)],
                                in1=xsl, op=OP.mult)
                            if ai > 0:
                                nc.vector.tensor_tensor(out=Y[:], in0=Y[:],
                                                        in1=t1[:], op=OP.add)
                        dstS = S if bi == 0 else t1
                        nc.vector.tensor_tensor(
                            out=dstS[:], in0=hEx[:, bass.ts(bi, FCH)],
                            in1=Y[:], op=OP.mult)
                        if bi > 0:
                            nc.vector.tensor_tensor(out=S[:], in0=S[:],
                                                    in1=t1[:], op=OP.add)
                    for s in range(CH // 2):
                        nc.tensor.matmul(pd[s][:], w_dc[:, bass.ts(k, C2)],
                                         S[:, bass.ts(s, SUB)],
                                         start=(k == 0), stop=(k == KK - 1))

                for s in range(CH // 2):
                    o1 = st3.tile([C2, SUB], F32, tag="o1")
                    nc.scalar.activation(o1[:], pd[s][:], AF.Relu,
                                         bias=b_dc[:, :])
                    fst = st3.tile([C2, SUB], F32, tag="fst")
                    base = (r0 + 2 * s) * PW
                    nc.sync.dma_start(out=fst[:],
                                      in_=farm32[:, base:base + SUB])
                    o2 = st3.tile([C2, SUB], F32, tag="o2")
                    nc.vector.tensor_tensor(out=o2[:], in0=o1[:], in1=fst[:],
                                            op=OP.add)
                    nc.sync.dma_start(out=out_pad[:, base:base + SUB],
                                      in_=o2[:])
    nc.compile()
    return nc


def _prep_inputs(inputs):
    feat_l = np.asarray(inputs['feat_l'], np.float32)
    feat_s = np.asarray(inputs['feat_s'], np.float32)
    watten = np.asarray(inputs['fsm_atten_w'], np.float32)
    wconv = np.asarray(inputs['fsm_conv_w'], np.float32)
    woff = np.asarray(inputs['offset_w'], np.float32)
    wom = np.asarray(inputs['dcn_om_w'], np.float32)
    omb = np.asarray(inputs['dcn_om_b'], np.float32)
    wdcn = np.asarray(inputs['dcn_w'], np.float32)
    dcnb = np.asarray(inputs['dcn_b'], np.float32)

    watten_T = np.ascontiguousarray((watten / (H * W)).T)
    wconv_T = np.ascontiguousarray(wconv.T)
    wofffa_T = np.ascontiguousarray(woff[:, :C2].T).astype(BF)
    wofffs_T = np.ascontiguousarray(woff[:, C2:].T * 2.0).astype(BF)

    perm = np.zeros(216, np.int64)
    for blk in range(3):
        for d in range(DG):
            for k in range(KK):
                perm[blk * 72 + k * 8 + d] = blk * 72 + d * 9 + k
    womp = wom[perm]
    wom_T = np.zeros((C2, 9 * 216), np.float32)
    for i in range(9):
        wom_T[:, i * 216:(i + 1) * 216] = womp[:, :, i // 3, i % 3].T
    ombp = omb[perm].reshape(216, 1)

    wdcn_T = np.zeros((C2, 9 * C2), np.float32)
    for k in range(KK):
        wdcn_T[:, k * C2:(k + 1) * C2] = wdcn[:, :, k // 3, k % 3].T

    common = {
        'watten': watten_T, 'wconv': wconv_T,
        'wofffa': wofffa_T, 'wofffs': wofffs_T,
        'wom': wom_T.astype(BF), 'wdcn': wdcn_T.astype(BF),
        'dcnb': dcnb.reshape(C2, 1), 'ombp': ombp,
    }

    maps = []
    for core in range(8):
        b, si = core // 4, core % 4
        h0 = si * SH
        xs = np.zeros((C2, XR, PW), np.float32)
        r_lo, r_hi = max(0, h0 - 4), min(H, h0 + 44)
        xs[:, r_lo - (h0 - 4):r_hi - (h0 - 4), 4:4 + W] = feat_s[b, :, r_lo:r_hi, :]
        xs1 = np.zeros((C2, XR, PW), np.float32)
        xs1[:, :, 1:] = xs[:, :, :-1]
        flx = np.zeros((C1, ER, W), np.float32)
        e_lo, e_hi = max(0, h0 - 1), min(H, h0 + 41)
        flx[:, e_lo - (h0 - 1):e_hi - (h0 - 1), :] = feat_l[b, :, e_lo:e_hi, :]
        m = dict(common)
        gs = np.zeros((C2, 4), np.float32)
        gs[:, b * 2:(b + 1) * 2] = 1.0
        m['gsel'] = gs
        m['xs0'] = xs.reshape(C2, XR * PW).astype(BF)
        m['xs1'] = xs1.reshape(C2, XR * PW).astype(BF)
        m['fl'] = flx.reshape(C1, ER * W)
        maps.append(m)
    return maps


def kernel(**inputs):
    if 'nc' not in _CACHE:
        _CACHE['nc'] = _build_program()
    nc = _CACHE['nc']
    maps = _prep_inputs(inputs)
    res = run_bass_kernel_spmd(nc, maps, list(range(8)))
    out = np.zeros((B, C2, H, W), np.float32)
    for core in range(8):
        b, si = core // 4, core % 4
        o = np.asarray(res.results[core]['out_pad']).reshape(C2, SH, PW)
        out[b, :, si * SH:(si + 1) * SH, :] = o[:, :, 4:4 + W]
    return out
